# revision 9
# baseline (speedup 1.0000x reference)
"""CapsuleLinear dynamic-routing kernel for TRN2 (8 NeuronCores, data-parallel over batch).

Math (reference):
    priors[n,j,i,k] = sum_l x[n,i,l] * w[j,k,l]          (never materialized: 151MB)
    3 routing iterations entirely in the L=8 compressed space:
      probs = softmax_j(logits)                          logits[n,i,j], init 0
      s[n,j,l]  = sum_i probs[n,j,i] * x[n,i,l]          (PE matmul, contraction over i)
      u[n,j,k]  = sum_l w[j,k,l] * s[n,j,l]              (DVE broadcast-mul + reduce)
      out       = squash_k(u)
      v[n,j,l]  = sum_k w[j,k,l] * out[n,j,k]            (DVE)
      logits   += sum_l x[n,i,l] * v[n,j,l]              (PE matmul, PSUM-resident accum)

Layout: i = 9*p + q  (p = SBUF partition 0..127, q = 0..8).
Softmax normalization is folded into x (xs = x * 1/d per i) so the wide (64-per-i)
probs tensor is never divided; sqrt is computed as exp(0.5*ln(.)) and Exp/Ln are
pinned to the single natural_log_exp ACT table set (no mid-kernel table reloads).
Matmul operands are bf16 (default; KPREC=f32 env reverts); PSUM accumulation f32.

v2 structure:
  - t=0 probs are uniform over j: s0[n,l] = (1/J) sum_i x[n,i,l], computed with one
    DVE q-reduce + two 1/J-valued-ones matmuls straight into the s2 PSUM blocks.
  - logits live in three PSUM tiles (one per softmax slab of 3 q's) so iteration
    t+1's exp of slab g starts as soon as t's three (b) matmuls for g finish.
  - v path runs from u (not oc) concurrently with the squash scale chain; the
    squash scale is applied to vT at the end; oc is materialized only at t=2.
"""

import os

import numpy as np

N, I, L, J, K = 32, 1152, 8, 64, 16
NCORES = 8
NPC = N // NCORES  # samples per core = 4
P = 128
Q = I // P  # 9
ITERS = 3
EPS = 1e-9

_cache = {}
LAST_RESULT = None


def _patch_act_tables():
    """Restrict Exp/Ln to the one table set containing both, so bacc's
    table-load pass never alternates sets (each reload costs ~2.7us)."""
    import concourse.hw_specs as hw_specs
    from concourse import mybir

    import concourse.bacc as bacc

    if getattr(hw_specs, "_capsule_patched", False):
        return
    orig = hw_specs.get_activation_tables

    def patched(arch):
        t = dict(orig(arch))
        AF = mybir.ActivationFunctionType
        both = "natural_log_exp_and_others"
        if both in t:
            for name in t:
                if name != both:
                    t[name] = t[name] - {AF.Exp, AF.Ln}
        return t

    hw_specs.get_activation_tables = patched
    bacc.get_activation_tables = patched  # bacc binds the name via from-import
    hw_specs._capsule_patched = True


def _build():
    import concourse.bacc as bacc
    import concourse.tile as tile
    from concourse import mybir
    from concourse.masks import make_identity

    _patch_act_tables()

    f32 = mybir.dt.float32
    bf16 = mybir.dt.bfloat16
    mode = os.environ.get("KPREC", "bf16")  # bf16 | f32
    mmdt = bf16 if mode == "bf16" else f32
    AF = mybir.ActivationFunctionType
    ALU = mybir.AluOpType
    AX = mybir.AxisListType

    nc = bacc.Bacc("TRN2", target_bir_lowering=False, debug=False, num_devices=NCORES)

    x_d = nc.dram_tensor("x", (NPC, I, L), f32, kind="ExternalInput")
    w_d = nc.dram_tensor("weight", (J, K, L), f32, kind="ExternalInput")
    o_d = nc.dram_tensor("out", (NPC, J, K), f32, kind="ExternalOutput")

    with tile.TileContext(nc) as tc:
        with tc.tile_pool(name="singles", bufs=1) as singles, \
             tc.tile_pool(name="work", bufs=2) as work:
            # ---- warm the ACT ln/exp table set immediately (table load ~2.6us
            # overlaps the input DMA instead of serializing after it) ----
            warm = singles.tile([1, 2], f32)
            nc.vector.memset(warm, 1.0)
            nc.scalar.activation(warm[:, 0:1], warm[:, 0:1], AF.Ln, bias=1.0)
            nc.scalar.activation(warm[:, 1:2], warm[:, 1:2], AF.Exp)
            eps_t = singles.tile([J, 1], f32)
            nc.vector.memset(eps_t, EPS)

            # ---- persistent SBUF tensors ----
            # xall[p, n, q, l] = x[n, 9p+q, l]; 288B contiguous runs; split in two
            # DMAs (different queues) to halve load latency
            xall = singles.tile([P, NPC, Q, L], f32)
            xsrc = x_d[:].rearrange("n (p q) l -> p n q l", p=P)
            nc.sync.dma_start(out=xall[:, 0:2], in_=xsrc[:, 0:2])
            nc.scalar.dma_start(out=xall[:, 2:4], in_=xsrc[:, 2:4])
            w_sb = singles.tile([J, K, L], f32)
            nc.gpsimd.dma_start(out=w_sb, in_=w_d[:])

            id_t = singles.tile([P, P], mmdt)
            make_identity(nc, id_t)
            # t=0 lhsT: every (n,j) column = 1/J, so s2 = (1/J) sum_p xqs
            colones = singles.tile([P, P], mmdt)
            nc.gpsimd.memset(colones, 1.0 / J)
            # block-diag v, padded: sample n occupies partitions 32n..32n+8
            # (compute-engine APs need 32-aligned base partitions)
            vblk = singles.tile([P, NPC * J], mmdt)
            nc.gpsimd.memset(vblk, 0.0)
            # padded v (pre-transpose): sample n in cols 32n..32n+8, zeros between
            vT_pad = singles.tile([J, NPC, 32], mmdt)
            nc.gpsimd.memset(vT_pad, 0.0)

            # w in bf16 (pt mul wants all-16-bit operands for DVE 2x mode) and
            # transposed+contiguous wT[j, l, k] for the v path
            w_b16 = singles.tile([J, K, L], mmdt)
            nc.vector.tensor_copy(w_b16, w_sb[:])
            wT_b16 = singles.tile([J, L, K], mmdt)
            nc.vector.tensor_copy(wT_b16, w_sb[:].transpose([0, 2, 1]))

            # padded + cast copy of x: [p, q, n, 32-pad] so one PE transpose per
            # q lands sample n at partitions 32n..32n+8; also the bf16 x source
            # for the xs fold (slices [.., 0:8])
            xpad = singles.tile([P, Q, NPC, 32], mmdt)
            nc.gpsimd.memset(xpad, 0.0)
            nc.vector.tensor_copy(
                xpad[:, :, :, 0:L], xall[:].transpose([0, 2, 1, 3])
            )

            # t=0 sum over q: xqs[p, n, l] = sum_q x[p, n, q, l]
            xqs = singles.tile([P, NPC, L], f32)
            nc.vector.tensor_reduce(
                xqs, xall[:].transpose([0, 1, 3, 2]), axis=AX.X, op=ALU.add
            )
            xqs_b16 = singles.tile([P, NPC, L], mmdt)
            nc.gpsimd.tensor_copy(xqs_b16, xqs[:])

            # ---- setup: xT[32n+l, q, p] = x[n, 9p+q, l] via 9 PE transposes ----
            xT_sb = singles.tile([P, Q, P], mmdt)
            with tc.tile_pool(name="setup_ps", bufs=2, space="PSUM") as setup_ps:
                for q in range(Q):
                    xT_ps = setup_ps.tile([P, P], mmdt)
                    nc.tensor.transpose(
                        xT_ps, xpad[:, q].rearrange("p n l -> p (n l)"), id_t
                    )
                    if q % 2 == 0:
                        nc.scalar.copy(xT_sb[:, q, :], xT_ps)
                    else:
                        nc.vector.tensor_copy(xT_sb[:, q, :], xT_ps)

            # logits: 3 PSUM tiles, one per softmax slab (3 q's = 1.5 banks
            # each).  Bank-clear rule: a matmul's start=True clears its whole
            # 2KB bank, and each [P, NPC*J] f32 q-slot is half a bank, so at
            # t=0 exactly the even global q's get start=True (adjacent tiles
            # included: tile boundaries fall mid-bank, covered by the same
            # parity rule since the 3 tiles are allocated back to back).
            with tc.tile_pool(name="logits_ps", bufs=1, space="PSUM") as logits_pool, \
                 tc.tile_pool(name="s2_ps", bufs=1, space="PSUM") as s2_pool, \
                 tc.tile_pool(name="vtr_ps", bufs=1, space="PSUM") as vtr_pool:
                logits_g = [
                    logits_pool.tile(
                        [P, 3, NPC, J], f32, tag=f"lg{g}", name=f"logits{g}"
                    )
                    for g in range(3)
                ]
                # s2a/s2b share one bank: only s2a's q=0 matmul uses start=True
                # (clearing the whole bank); s2b accumulates onto the cleared
                # half without its own start.
                s2 = s2_pool.tile([P, 2, 2 * L], f32, tag="s2")

                oc = None
                for t in range(ITERS):
                    # ---------- probs folded into xs = x/sum_j exp(logits) ----------
                    if t == 0:
                        # uniform probs: s[n,j,l] = (1/J) sum_i x[n,i,l] for all j
                        for h in range(2):
                            nc.tensor.matmul(
                                s2[:, h],
                                colones[:],
                                xqs_b16[:, 2 * h : 2 * h + 2].rearrange(
                                    "p n l -> p (n l)"
                                ),
                                start=(h == 0),
                                stop=(h == 1),
                                skip_group_check=True,
                            )
                        e_t = None
                    else:
                        # slab-grouped softmax: exp of slab g depends only on
                        # logits_g, i.e. on 3 of the previous iteration's 9 (b)
                        # matmuls -> overlaps with the rest of (b)
                        e_t = work.tile([P, Q, NPC, J], mmdt, tag="e")
                        d_t = work.tile([P, Q, NPC], f32, tag="d")
                        r_t = work.tile([P, Q, NPC], f32, tag="r")
                        xs = work.tile([P, Q, NPC, L], mmdt, tag="xs")
                        for g in range(3):
                            sl = slice(3 * g, 3 * g + 3)
                            nc.scalar.activation(
                                e_t[:, sl].rearrange("p q n j -> p (q n j)"),
                                logits_g[g][:].rearrange("p q n j -> p (q n j)"),
                                AF.Exp,
                            )
                            nc.vector.tensor_reduce(
                                d_t[:, sl],
                                e_t[:, sl],
                                axis=AX.X,
                                op=ALU.add,
                            )
                            nc.vector.reciprocal(r_t[:, sl], d_t[:, sl])
                            nc.gpsimd.tensor_mul(
                                xs[:, sl],
                                xpad[:, sl, :, 0:L],
                                r_t[:, sl].unsqueeze(3).broadcast_to((P, 3, NPC, L)),
                            )
                            # ---------- (a): s2[h][(nh,j),(nh',l)] = sum_i e*xs ----------
                            for qq in range(3 * g, 3 * g + 3):
                                for h in range(2):
                                    nc.tensor.matmul(
                                        s2[:, h],
                                        e_t[:, qq, 2 * h : 2 * h + 2, :].rearrange(
                                            "p n j -> p (n j)"
                                        ),
                                        xs[:, qq, 2 * h : 2 * h + 2, :].rearrange(
                                            "p n l -> p (n l)"
                                        ),
                                        start=(qq == 0 and h == 0),
                                        stop=(qq == Q - 1 and h == 1),
                                        skip_group_check=True,
                                    )

                    # extract diag blocks -> s_sb[j, n, l] (bf16 for DVE 2x)
                    s_sb = work.tile([J, NPC, L], mmdt, tag="s_sb")
                    eng = [nc.scalar.copy, nc.vector.tensor_copy,
                           nc.scalar.copy, nc.vector.tensor_copy]
                    for n in range(NPC):
                        h, n2 = divmod(n, 2)
                        src = s2[64 * n2 : 64 * n2 + 64, h, 8 * n2 : 8 * n2 + 8]
                        eng[n](s_sb[:, n, :], src)

                    # ---------- u[j, n, k] = sum_l w[j,k,l] * s[j,n,l] ----------
                    pt = work.tile([J, NPC, K, L], mmdt, tag="pt")
                    nc.vector.tensor_mul(
                        pt,
                        w_b16[:].unsqueeze(1).broadcast_to((J, NPC, K, L)),
                        s_sb[:].unsqueeze(2).broadcast_to((J, NPC, K, L)),
                    )
                    u_t = work.tile([J, NPC, K], f32, tag="u")
                    nc.vector.reduce_sum(u_t, pt, axis=AX.X)

                    # ---------- v path from u (runs while squash scale computes):
                    # vT_u[j,n,l] = sum_k wT[j,l,k] * u[j,n,k]
                    if t != ITERS - 1:
                        qt = work.tile([J, NPC, L, K], mmdt, tag="qt")
                        nc.vector.tensor_mul(
                            qt,
                            wT_b16[:].unsqueeze(1).broadcast_to((J, NPC, L, K)),
                            u_t[:].unsqueeze(2).broadcast_to((J, NPC, L, K)),
                        )
                        vT_u = work.tile([J, NPC, L], f32, tag="vT_u")
                        nc.vector.reduce_sum(vT_u, qt, axis=AX.X)

                    # ---------- squash scale: scl = sq/((1+sq)*sqrt(sq+eps)) ----------
                    u2 = work.tile([J, NPC, K], f32, tag="u2")
                    nc.gpsimd.tensor_mul(u2, u_t, u_t)
                    sq = work.tile([J, NPC], f32, tag="sq")
                    nc.vector.reduce_sum(sq, u2, axis=AX.X)
                    rt = work.tile([J, NPC], f32, tag="rt")  # sqrt(sq+eps)
                    nc.scalar.activation(rt, sq, AF.Ln, bias=eps_t[:])
                    nc.scalar.activation(rt, rt, AF.Exp, scale=0.5)
                    sp1 = work.tile([J, NPC], f32, tag="sp1")  # (1+sq)*sqrt(sq+eps)
                    nc.vector.scalar_tensor_tensor(
                        sp1, sq, 1.0, rt, op0=ALU.add, op1=ALU.mult
                    )
                    nc.vector.reciprocal(sp1, sp1)
                    scl = work.tile([J, NPC], f32, tag="scl")
                    nc.vector.tensor_mul(scl, sq, sp1)

                    if t == ITERS - 1:
                        oc = work.tile([J, NPC, K], f32, tag="oc")
                        nc.vector.tensor_mul(
                            oc, u_t, scl[:].unsqueeze(2).broadcast_to((J, NPC, K))
                        )
                        break

                    # ---------- v = scl * vT_u, straight into the padded layout ----------
                    nc.vector.tensor_mul(
                        vT_pad[:, :, 0:L],
                        vT_u[:],
                        scl[:].unsqueeze(2).broadcast_to((J, NPC, L)),
                    )

                    # one PE transpose: (64, 128) -> (128, 64); sample n lands at
                    # partitions 32n..32n+8 (32-aligned, so copies below are legal)
                    vtr = vtr_pool.tile([P, J], mmdt, tag="vtr")
                    nc.tensor.transpose(
                        vtr, vT_pad[:].rearrange("j n l -> j (n l)"), id_t[:J, :J]
                    )
                    veng = [nc.scalar.copy, nc.vector.tensor_copy,
                            nc.scalar.copy, nc.vector.tensor_copy]
                    for n in range(NPC):
                        src = vtr[32 * n : 32 * n + 8, :]
                        dst = vblk[32 * n : 32 * n + 8, 64 * n : 64 * n + 64]
                        veng[n](dst, src)

                    # ---------- (b): logits_g[p, s, n, j] += sum_l x * v ----------
                    # each logits tile is bank-aligned (2 banks, PSUM alloc is
                    # bank-granular); slots s=0,1 share the tile's first bank,
                    # s=2 leads its second -> start=True on even s at t=0
                    for g in range(3):
                        for s in range(3):
                            qq = 3 * g + s
                            nc.tensor.matmul(
                                logits_g[g][:, s].rearrange("p n j -> p (n j)"),
                                xT_sb[:, qq, :],
                                vblk[:],
                                start=(t == 0 and s % 2 == 0),
                                stop=(t == ITERS - 2 and s >= 1),
                                skip_group_check=True,
                            )

                # ---------- output ----------
                nc.sync.dma_start(out=o_d[:].transpose([1, 0, 2]), in_=oc)

    nc.finalize()
    return nc


def kernel(x, weight):
    global LAST_RESULT
    from concourse.bass_utils import run_bass_kernel_spmd

    if "nc" not in _cache:
        _cache["nc"] = _build()
    nc = _cache["nc"]

    x = np.ascontiguousarray(np.asarray(x, dtype=np.float32))
    weight = np.ascontiguousarray(np.asarray(weight, dtype=np.float32))

    in_maps = [
        {"x": x[c * NPC : (c + 1) * NPC], "weight": weight} for c in range(NCORES)
    ]
    last_exc = None
    for attempt in range(3):
        try:
            res = run_bass_kernel_spmd(nc, in_maps, core_ids=list(range(NCORES)))
            break
        except Exception as e:
            last_exc = e
            import time

            time.sleep(5 * (attempt + 1))
    else:
        raise last_exc
    LAST_RESULT = res
    return np.concatenate([r["out"] for r in res.results], axis=0)


# revision 14
# speedup vs baseline: 1.0546x; 1.0546x over previous
"""CapsuleLinear dynamic-routing kernel for TRN2 (8 NeuronCores, data-parallel over batch).

Math (reference):
    priors[n,j,i,k] = sum_l x[n,i,l] * w[j,k,l]          (never materialized: 151MB)
    3 routing iterations entirely in the L=8 compressed space:
      probs = softmax_j(logits)                          logits[n,i,j], init 0
      s[n,j,l]  = sum_i probs[n,j,i] * x[n,i,l]          (PE matmul, contraction over i)
      u[n,j,k]  = sum_l w[j,k,l] * s[n,j,l]              (DVE broadcast-mul + reduce)
      out       = squash_k(u)
      v[n,j,l]  = sum_k w[j,k,l] * out[n,j,k]            (DVE)
      logits   += sum_l x[n,i,l] * v[n,j,l]              (PE matmul, PSUM-resident accum)

Layout: i = 9*p + q  (p = SBUF partition 0..127, q = 0..8).
Softmax normalization is folded into x (xs = x * 1/d per i) so the wide (64-per-i)
probs tensor is never divided; sqrt is computed as exp(0.5*ln(.)) and Exp/Ln are
pinned to the single natural_log_exp ACT table set (no mid-kernel table reloads).
Matmul operands are bf16 (default; KPREC=f32 env reverts); PSUM accumulation f32.

v3 structure:
  - Samples are processed in the permuted order n' = [0, 2, 1, 3] everywhere
    (xpad/xT/vblk/logits/e/xs).  The (a)-matmul pair h then covers samples
    {h, h+2}, so the per-sample diag blocks of the two s2 halves stack into a
    SINGLE [128 = (m j), h, l] tile with just 2 same-partition copies, and the
    whole mid-chain (u / squash / v) runs on 128 partitions with half the
    per-partition free size of the 64-partition version.
  - v is placed into a [128 (m j), (n',32)] padded tile (2 writes, one per m
    half); one PE transpose then lands every sample at vtr[32n'+l, 64m+j],
    making all 4 vblk copies same-partition.
  - t=0 probs are uniform over j: s2 comes from two 1/J-ones matmuls against
    the q-summed x, skipping softmax and the 18 (a) matmuls.
  - logits live in three PSUM tiles (one per softmax slab of 3 q's) so
    iteration t+1's exp of slab g starts after only 3 of t's 9 (b) matmuls.
  - reciprocals use the fast custom-DVE approximation (~18 bits, plenty for
    the 2e-2 gate).
"""

import os

import numpy as np

N, I, L, J, K = 32, 1152, 8, 64, 16
NCORES = 8
NPC = N // NCORES  # samples per core = 4
P = 128
Q = I // P  # 9
ITERS = 3
EPS = 1e-9

_cache = {}
LAST_RESULT = None


def _patch_act_tables():
    """Restrict Exp/Ln to the one table set containing both, so bacc's
    table-load pass never alternates sets (each reload costs ~2.7us)."""
    import concourse.hw_specs as hw_specs
    from concourse import mybir

    import concourse.bacc as bacc

    if getattr(hw_specs, "_capsule_patched", False):
        return
    orig = hw_specs.get_activation_tables

    def patched(arch):
        t = dict(orig(arch))
        AF = mybir.ActivationFunctionType
        both = "natural_log_exp_and_others"
        if both in t:
            for name in t:
                if name != both:
                    t[name] = t[name] - {AF.Exp, AF.Ln}
        return t

    hw_specs.get_activation_tables = patched
    bacc.get_activation_tables = patched  # bacc binds the name via from-import
    hw_specs._capsule_patched = True


def _build():
    import concourse.bacc as bacc
    import concourse.tile as tile
    from concourse import mybir
    from concourse.masks import make_identity

    _patch_act_tables()

    f32 = mybir.dt.float32
    bf16 = mybir.dt.bfloat16
    mode = os.environ.get("KPREC", "bf16")  # bf16 | f32
    mmdt = bf16 if mode == "bf16" else f32
    AF = mybir.ActivationFunctionType
    ALU = mybir.AluOpType
    AX = mybir.AxisListType

    nc = bacc.Bacc("TRN2", target_bir_lowering=False, debug=False, num_devices=NCORES)

    x_d = nc.dram_tensor("x", (NPC, I, L), f32, kind="ExternalInput")
    w_d = nc.dram_tensor("weight", (J, K, L), f32, kind="ExternalInput")
    o_d = nc.dram_tensor("out", (NPC, J, K), f32, kind="ExternalOutput")

    with tile.TileContext(nc) as tc:
        with tc.tile_pool(name="singles", bufs=1) as singles, \
             tc.tile_pool(name="work", bufs=2) as work:
            # ---- warm the ACT ln/exp table set immediately (table load ~2.6us
            # overlaps the input DMA instead of serializing after it) ----
            warm = singles.tile([1, 2], f32)
            nc.vector.memset(warm, 1.0)
            nc.scalar.activation(warm[:, 0:1], warm[:, 0:1], AF.Ln, bias=1.0)
            nc.scalar.activation(warm[:, 1:2], warm[:, 1:2], AF.Exp)
            eps_t = singles.tile([P, 1], f32)
            nc.vector.memset(eps_t, EPS)

            # padded + cast x in the n' sample order: xpad[p, q, n', 32-pad]
            # (memset on vector, overlapping the input DMA)
            xpad = singles.tile([P, Q, NPC, 32], mmdt)
            nc.vector.memset(xpad, 0.0)

            # ---- inputs ----
            # xall[p, n, q, l] = x[n, 9p+q, l]; 288B contiguous runs; split in two
            # DMAs (different queues) to halve load latency
            xall = singles.tile([P, NPC, Q, L], f32)
            xsrc = x_d[:].rearrange("n (p q) l -> p n q l", p=P)
            nc.sync.dma_start(out=xall[:, 0:2], in_=xsrc[:, 0:2])
            nc.scalar.dma_start(out=xall[:, 2:4], in_=xsrc[:, 2:4])
            # w replicated on both partition halves (rows 64m+j)
            w_f = singles.tile([P, K, L], f32)
            nc.sync.dma_start(out=w_f[0:J], in_=w_d[:])
            nc.gpsimd.dma_start(out=w_f[J:P], in_=w_d[:])

            id_t = singles.tile([P, P], mmdt)
            make_identity(nc, id_t)
            # t=0 lhsT: every (n,j) column = 1/J, so s2 = (1/J) sum_p xqs
            colones = singles.tile([P, P], mmdt)
            nc.gpsimd.memset(colones, 1.0 / J)
            # block-diag v, padded: sample n' occupies partitions 32n'..32n'+8
            vblk = singles.tile([P, NPC * J], mmdt)
            nc.gpsimd.memset(vblk, 0.0)
            # padded v (pre-transpose): row (m,j), sample n'=2h+m at cols
            # 32n'..32n'+8 of its own row half; zeros elsewhere
            vT_pad = singles.tile([P, NPC, 32], mmdt)
            nc.gpsimd.memset(vT_pad, 0.0)

            # w in bf16 (all-16-bit operands get DVE 2x) + transposed wT[.,l,k]
            w_b = singles.tile([P, K, L], mmdt)
            nc.gpsimd.tensor_copy(w_b, w_f[:])
            wT_b = singles.tile([P, L, K], mmdt)
            nc.gpsimd.tensor_copy(wT_b, w_f[:].transpose([0, 2, 1]))

            # t=0 sum over q: xqs[p, n', l], already in n' order
            xqs = singles.tile([P, NPC, L], f32)
            nc.vector.tensor_reduce(
                xqs[:].rearrange("p (a c) l -> p a c l", a=2),
                xall[:].rearrange("p (c a) q l -> p a c l q", c=2),
                axis=AX.X,
                op=ALU.add,
            )
            # n' sample order: n = (c a) -> n' = (a c)
            nc.vector.tensor_copy(
                xpad[:, :, :, 0:L].rearrange("p q (a c) l -> p q a c l", a=2),
                xall[:].rearrange("p (c a) q l -> p q a c l", c=2),
            )
            xqs_b = singles.tile([P, NPC, L], mmdt)
            nc.gpsimd.tensor_copy(xqs_b, xqs[:])

            # ---- setup: xT[32n'+l, q, p] via 9 PE transposes ----
            xT_sb = singles.tile([P, Q, P], mmdt)
            with tc.tile_pool(name="setup_ps", bufs=2, space="PSUM") as setup_ps:
                for q in range(Q):
                    xT_ps = setup_ps.tile([P, P], mmdt)
                    nc.tensor.transpose(
                        xT_ps, xpad[:, q].rearrange("p n l -> p (n l)"), id_t
                    )
                    if q % 2 == 0:
                        nc.scalar.copy(xT_sb[:, q, :], xT_ps)
                    else:
                        nc.vector.tensor_copy(xT_sb[:, q, :], xT_ps)

            # logits: 3 PSUM tiles, one per softmax slab (3 q's; 2 banks each,
            # PSUM alloc is bank-granular).  Slots s=0,1 share the tile's
            # first bank, s=2 leads its second -> start=True on even s at t=0
            # (start clears the whole 2KB bank).
            with tc.tile_pool(name="logits_ps", bufs=1, space="PSUM") as logits_pool, \
                 tc.tile_pool(name="s2_ps", bufs=1, space="PSUM") as s2_pool, \
                 tc.tile_pool(name="vtr_ps", bufs=1, space="PSUM") as vtr_pool:
                logits_g = [
                    logits_pool.tile(
                        [P, 3, NPC, J], f32, tag=f"lg{g}", name=f"logits{g}"
                    )
                    for g in range(3)
                ]
                # s2a/s2b share one bank: only s2a's first matmul uses
                # start=True (clearing the whole bank); s2b accumulates onto
                # the cleared half without its own start.
                s2 = s2_pool.tile([P, 2, 2 * L], f32, tag="s2")

                oc = None
                for t in range(ITERS):
                    # ---------- probs folded into xs = x/sum_j exp(logits) ----------
                    if t == 0:
                        # uniform probs: s[n,j,l] = (1/J) sum_i x[n,i,l] for all j
                        for h in range(2):
                            nc.tensor.matmul(
                                s2[:, h],
                                colones[:],
                                xqs_b[:, 2 * h : 2 * h + 2].rearrange(
                                    "p n l -> p (n l)"
                                ),
                                start=(h == 0),
                                stop=(h == 1),
                                skip_group_check=True,
                            )
                    else:
                        # slab-grouped softmax: exp of slab g depends only on
                        # logits_g, i.e. on 3 of the previous iteration's 9 (b)
                        # matmuls -> overlaps with the rest of (b)
                        e_t = work.tile([P, Q, NPC, J], mmdt, tag="e")
                        d_t = work.tile([P, Q, NPC], f32, tag="d")
                        r_t = work.tile([P, Q, NPC], f32, tag="r")
                        xs = work.tile([P, Q, NPC, L], mmdt, tag="xs")
                        for g in range(3):
                            sl = slice(3 * g, 3 * g + 3)
                            nc.scalar.activation(
                                e_t[:, sl].rearrange("p q n j -> p (q n j)"),
                                logits_g[g][:].rearrange("p q n j -> p (q n j)"),
                                AF.Exp,
                            )
                            nc.vector.tensor_reduce(
                                d_t[:, sl],
                                e_t[:, sl],
                                axis=AX.X,
                                op=ALU.add,
                            )
                            nc.vector.reciprocal_approx_fast(r_t[:, sl], d_t[:, sl])
                            nc.gpsimd.tensor_mul(
                                xs[:, sl],
                                xpad[:, sl, :, 0:L],
                                r_t[:, sl].unsqueeze(3).broadcast_to((P, 3, NPC, L)),
                            )
                            # ------- (a): s2[h][(nh,j),(nh',l)] = sum_i e*xs -------
                            for qq in range(3 * g, 3 * g + 3):
                                for h in range(2):
                                    nc.tensor.matmul(
                                        s2[:, h],
                                        e_t[:, qq, 2 * h : 2 * h + 2, :].rearrange(
                                            "p n j -> p (n j)"
                                        ),
                                        xs[:, qq, 2 * h : 2 * h + 2, :].rearrange(
                                            "p n l -> p (n l)"
                                        ),
                                        start=(qq == 0 and h == 0),
                                        stop=(qq == Q - 1 and h == 1),
                                        skip_group_check=True,
                                    )

                    # extract diag blocks -> s_sb[(m j), h, l]: sample 2m+h
                    # (two same-partition copies; bf16 for DVE 2x)
                    s_sb = work.tile([P, 2, L], mmdt, tag="s_sb")
                    nc.scalar.copy(s_sb[0:J], s2[0:J, :, 0:L])
                    nc.vector.tensor_copy(s_sb[J:P], s2[J:P, :, L : 2 * L])

                    # ---------- u[(m j), h, k] = sum_l w[j,k,l] * s[(m j),h,l] ----------
                    pt = work.tile([P, 2, K, L], mmdt, tag="pt")
                    nc.vector.tensor_mul(
                        pt,
                        w_b[:].unsqueeze(1).broadcast_to((P, 2, K, L)),
                        s_sb[:].unsqueeze(2).broadcast_to((P, 2, K, L)),
                    )
                    u_t = work.tile([P, 2, K], f32, tag="u")
                    nc.vector.reduce_sum(u_t, pt, axis=AX.X)

                    # ---------- v path from u (runs while squash scale computes):
                    # vT_u[(m j), h, l] = sum_k wT[j,l,k] * u[(m j),h,k]
                    if t != ITERS - 1:
                        qt = work.tile([P, 2, L, K], mmdt, tag="qt")
                        nc.vector.tensor_mul(
                            qt,
                            wT_b[:].unsqueeze(1).broadcast_to((P, 2, L, K)),
                            u_t[:].unsqueeze(2).broadcast_to((P, 2, L, K)),
                        )
                        vT_u = work.tile([P, 2, L], f32, tag="vT_u")
                        nc.vector.reduce_sum(vT_u, qt, axis=AX.X)

                    # ---------- squash scale: scl = sq/((1+sq)*sqrt(sq+eps)) ----------
                    u2 = work.tile([P, 2, K], f32, tag="u2")
                    nc.vector.tensor_mul(u2, u_t, u_t)
                    sq = work.tile([P, 2], f32, tag="sq")
                    nc.vector.reduce_sum(sq, u2, axis=AX.X)
                    rt = work.tile([P, 2], f32, tag="rt")  # sqrt(sq+eps)
                    nc.scalar.activation(rt, sq, AF.Ln, bias=eps_t[:])
                    nc.scalar.activation(rt, rt, AF.Exp, scale=0.5)
                    sp1 = work.tile([P, 2], f32, tag="sp1")  # (1+sq)*sqrt(sq+eps)
                    nc.vector.scalar_tensor_tensor(
                        sp1, sq, 1.0, rt, op0=ALU.add, op1=ALU.mult
                    )
                    rp = work.tile([P, 2], f32, tag="rp")
                    nc.vector.reciprocal_approx_fast(rp, sp1)
                    scl = work.tile([P, 2], f32, tag="scl")
                    nc.vector.tensor_mul(scl, sq, rp)

                    if t == ITERS - 1:
                        oc = work.tile([P, 2, K], f32, tag="oc")
                        nc.vector.tensor_mul(
                            oc, u_t, scl[:].unsqueeze(2).broadcast_to((P, 2, K))
                        )
                        break

                    # ---------- v = scl * vT_u into the padded layout ----------
                    # row half m holds samples n' = {m, m+2} at col slots
                    # 32n'..32n'+8 -> strided slot AP, one write per m
                    vpv = vT_pad[:].rearrange("p (h m) x -> p m h x", h=2)
                    for m in range(2):
                        nc.vector.tensor_mul(
                            vpv[J * m : J * (m + 1), m, :, 0:L],
                            vT_u[J * m : J * (m + 1)],
                            scl[J * m : J * (m + 1)]
                            .unsqueeze(2)
                            .broadcast_to((J, 2, L)),
                        )

                    # one PE transpose: sample n'=2h+m lands at
                    # vtr[32n'+l, 64m+j] -- all 4 vblk copies same-partition
                    vtr = vtr_pool.tile([P, P], mmdt, tag="vtr")
                    nc.tensor.transpose(
                        vtr, vT_pad[:].rearrange("p n x -> p (n x)"), id_t
                    )
                    veng = [nc.scalar.copy, nc.vector.tensor_copy,
                            nc.scalar.copy, nc.vector.tensor_copy]
                    for np_ in range(NPC):
                        m = np_ % 2
                        src = vtr[32 * np_ : 32 * np_ + 8, J * m : J * (m + 1)]
                        dst = vblk[32 * np_ : 32 * np_ + 8, J * np_ : J * (np_ + 1)]
                        veng[np_](dst, src)

                    # ---------- (b): logits_g[p, s, n', j] += sum_l x * v ----------
                    for g in range(3):
                        for s in range(3):
                            qq = 3 * g + s
                            nc.tensor.matmul(
                                logits_g[g][:, s].rearrange("p n j -> p (n j)"),
                                xT_sb[:, qq, :],
                                vblk[:],
                                start=(t == 0 and s % 2 == 0),
                                stop=(t == ITERS - 2 and s >= 1),
                                skip_group_check=True,
                            )

                # ---------- output: o_d[n=2m+h, j, k] = oc[(m j), h, k] ----------
                # sample n = 2m + h lives at oc[64m + j, h, k]
                nc.sync.dma_start(
                    out=o_d[0:2].rearrange("h j k -> j h k"), in_=oc[0:J]
                )
                nc.scalar.dma_start(
                    out=o_d[2:4].rearrange("h j k -> j h k"), in_=oc[J:P]
                )

    nc.finalize()
    return nc


def kernel(x, weight):
    global LAST_RESULT
    from concourse.bass_utils import run_bass_kernel_spmd

    if "nc" not in _cache:
        _cache["nc"] = _build()
    nc = _cache["nc"]

    x = np.ascontiguousarray(np.asarray(x, dtype=np.float32))
    weight = np.ascontiguousarray(np.asarray(weight, dtype=np.float32))

    in_maps = [
        {"x": x[c * NPC : (c + 1) * NPC], "weight": weight} for c in range(NCORES)
    ]
    last_exc = None
    for attempt in range(3):
        try:
            res = run_bass_kernel_spmd(nc, in_maps, core_ids=list(range(NCORES)))
            break
        except Exception as e:
            last_exc = e
            import time

            time.sleep(5 * (attempt + 1))
    else:
        raise last_exc
    LAST_RESULT = res
    return np.concatenate([r["out"] for r in res.results], axis=0)


# revision 17
# speedup vs baseline: 1.0742x; 1.0187x over previous
"""CapsuleLinear dynamic-routing kernel for TRN2 (8 NeuronCores, data-parallel over batch).

Math (reference):
    priors[n,j,i,k] = sum_l x[n,i,l] * w[j,k,l]          (never materialized: 151MB)
    3 routing iterations entirely in the L=8 compressed space:
      probs = softmax_j(logits)                          logits[n,i,j], init 0
      s[n,j,l]  = sum_i probs[n,j,i] * x[n,i,l]          (PE matmul, contraction over i)
      u[n,j,k]  = sum_l w[j,k,l] * s[n,j,l]              (DVE broadcast-mul + reduce)
      out       = squash_k(u)
      v[n,j,l]  = sum_k w[j,k,l] * out[n,j,k]            (DVE)
      logits   += sum_l x[n,i,l] * v[n,j,l]              (PE matmul, PSUM-resident accum)

Layout: i = 9*p + q  (p = SBUF partition 0..127, q = 0..8).
Softmax normalization is folded into x (xs = x * 1/d per i) so the wide (64-per-i)
probs tensor is never divided; sqrt is computed as exp(0.5*ln(.)) and Exp/Ln are
pinned to the single natural_log_exp ACT table set (no mid-kernel table reloads).
Matmul operands are bf16 (default; KPREC=f32 env reverts); PSUM accumulation f32.

v3 structure:
  - Samples are processed in the permuted order n' = [0, 2, 1, 3] everywhere
    (xpad/xT/vblk/logits/e/xs).  The (a)-matmul pair h then covers samples
    {h, h+2}, so the per-sample diag blocks of the two s2 halves stack into a
    SINGLE [128 = (m j), h, l] tile with just 2 same-partition copies, and the
    whole mid-chain (u / squash / v) runs on 128 partitions with half the
    per-partition free size of the 64-partition version.
  - v is placed into a [128 (m j), (n',32)] padded tile (2 writes, one per m
    half); one PE transpose then lands every sample at vtr[32n'+l, 64m+j],
    making all 4 vblk copies same-partition.
  - t=0 probs are uniform over j: s2 comes from two 1/J-ones matmuls against
    the q-summed x, skipping softmax and the 18 (a) matmuls.
  - logits live in three PSUM tiles (one per softmax slab of 3 q's) so
    iteration t+1's exp of slab g starts after only 3 of t's 9 (b) matmuls.
  - reciprocals use the fast custom-DVE approximation (~18 bits, plenty for
    the 2e-2 gate).
"""

import os

import numpy as np

N, I, L, J, K = 32, 1152, 8, 64, 16
NCORES = 8
NPC = N // NCORES  # samples per core = 4
P = 128
Q = I // P  # 9
ITERS = 3
EPS = 1e-9

_cache = {}
LAST_RESULT = None


def _patch_act_tables():
    """Restrict Exp/Ln to the one table set containing both, so bacc's
    table-load pass never alternates sets (each reload costs ~2.7us)."""
    import concourse.hw_specs as hw_specs
    from concourse import mybir

    import concourse.bacc as bacc

    if getattr(hw_specs, "_capsule_patched", False):
        return
    orig = hw_specs.get_activation_tables

    def patched(arch):
        t = dict(orig(arch))
        AF = mybir.ActivationFunctionType
        both = "natural_log_exp_and_others"
        if both in t:
            for name in t:
                if name != both:
                    t[name] = t[name] - {AF.Exp, AF.Ln}
        return t

    hw_specs.get_activation_tables = patched
    bacc.get_activation_tables = patched  # bacc binds the name via from-import
    hw_specs._capsule_patched = True


def _build():
    import concourse.bacc as bacc
    import concourse.tile as tile
    from concourse import mybir
    from concourse.masks import make_identity

    _patch_act_tables()

    f32 = mybir.dt.float32
    bf16 = mybir.dt.bfloat16
    mode = os.environ.get("KPREC", "bf16")  # bf16 | f32
    mmdt = bf16 if mode == "bf16" else f32
    AF = mybir.ActivationFunctionType
    ALU = mybir.AluOpType
    AX = mybir.AxisListType

    nc = bacc.Bacc("TRN2", target_bir_lowering=False, debug=False, num_devices=NCORES)

    x_d = nc.dram_tensor("x", (NPC, I, L), f32, kind="ExternalInput")
    w_d = nc.dram_tensor("weight", (J, K, L), f32, kind="ExternalInput")
    o_d = nc.dram_tensor("out", (NPC, J, K), f32, kind="ExternalOutput")

    with tile.TileContext(nc) as tc:
        with tc.tile_pool(name="singles", bufs=1) as singles, \
             tc.tile_pool(name="work", bufs=2) as work:
            # ---- warm the ACT ln/exp table set immediately (table load ~2.6us
            # overlaps the input DMA instead of serializing after it) ----
            warm = singles.tile([1, 2], f32)
            nc.vector.memset(warm, 1.0)
            nc.scalar.activation(warm[:, 0:1], warm[:, 0:1], AF.Ln, bias=1.0)
            nc.scalar.activation(warm[:, 1:2], warm[:, 1:2], AF.Exp)
            eps_t = singles.tile([P, 1], f32)
            nc.vector.memset(eps_t, EPS)

            # padded + cast x in the n' sample order: xpad[p, q, n', 32-pad]
            # (memset on vector, overlapping the input DMA)
            xpad = singles.tile([P, Q, NPC, 32], mmdt)
            nc.vector.memset(xpad, 0.0)

            # ---- inputs ----
            # xall[p, n, q, l] = x[n, 9p+q, l]; 288B contiguous runs; split in two
            # DMAs (different queues) to halve load latency
            xall = singles.tile([P, NPC, Q, L], f32)
            xsrc = x_d[:].rearrange("n (p q) l -> p n q l", p=P)
            nc.sync.dma_start(out=xall[:, 0:2], in_=xsrc[:, 0:2])
            nc.scalar.dma_start(out=xall[:, 2:4], in_=xsrc[:, 2:4])
            # w replicated on both partition halves (rows 64m+j)
            w_f = singles.tile([P, K, L], f32)
            nc.sync.dma_start(out=w_f[0:J], in_=w_d[:])
            nc.gpsimd.dma_start(out=w_f[J:P], in_=w_d[:])

            id_t = singles.tile([P, P], mmdt)
            make_identity(nc, id_t)
            # t=0 lhsT: every (n,j) column = 1/J, so s2 = (1/J) sum_p xqs
            colones = singles.tile([P, P], mmdt)
            nc.gpsimd.memset(colones, 1.0 / J)
            # block-diag v, padded: sample n' occupies partitions 32n'..32n'+8
            vblk = singles.tile([P, NPC * J], mmdt)
            nc.gpsimd.memset(vblk, 0.0)
            # padded v (pre-transpose): row (m,j), sample n'=2h+m at cols
            # 32n'..32n'+8 of its own row half; zeros elsewhere
            vT_pad = singles.tile([P, NPC, 32], mmdt)
            nc.gpsimd.memset(vT_pad, 0.0)

            # w in bf16 (all-16-bit operands get DVE 2x) + transposed wT[.,l,k]
            w_b = singles.tile([P, K, L], mmdt)
            nc.gpsimd.tensor_copy(w_b, w_f[:])
            wT_b = singles.tile([P, L, K], mmdt)
            nc.gpsimd.tensor_copy(wT_b, w_f[:].transpose([0, 2, 1]))

            # t=0 sum over q: xqs[p, n', l], already in n' order
            xqs = singles.tile([P, NPC, L], f32)
            nc.vector.tensor_reduce(
                xqs[:].rearrange("p (a c) l -> p a c l", a=2),
                xall[:].rearrange("p (c a) q l -> p a c l q", c=2),
                axis=AX.X,
                op=ALU.add,
            )
            # n' sample order: n = (c a) -> n' = (a c)
            nc.vector.tensor_copy(
                xpad[:, :, :, 0:L].rearrange("p q (a c) l -> p q a c l", a=2),
                xall[:].rearrange("p (c a) q l -> p q a c l", c=2),
            )
            xqs_b = singles.tile([P, NPC, L], mmdt)
            nc.gpsimd.tensor_copy(xqs_b, xqs[:])

            # ---- setup: xT[32n'+l, q, p] via 9 PE transposes ----
            xT_sb = singles.tile([P, Q, P], mmdt)
            with tc.tile_pool(name="setup_ps", bufs=2, space="PSUM") as setup_ps:
                for q in range(Q):
                    xT_ps = setup_ps.tile([P, P], mmdt)
                    nc.tensor.transpose(
                        xT_ps, xpad[:, q].rearrange("p n l -> p (n l)"), id_t
                    )
                    if q % 2 == 0:
                        nc.scalar.copy(xT_sb[:, q, :], xT_ps)
                    else:
                        nc.vector.tensor_copy(xT_sb[:, q, :], xT_ps)

            # logits: 3 PSUM tiles, one per softmax slab (3 q's; 2 banks each,
            # PSUM alloc is bank-granular).  Slots s=0,1 share the tile's
            # first bank, s=2 leads its second -> start=True on even s at t=0
            # (start clears the whole 2KB bank).
            with tc.tile_pool(name="logits_ps", bufs=1, space="PSUM") as logits_pool, \
                 tc.tile_pool(name="s2_ps", bufs=1, space="PSUM") as s2_pool, \
                 tc.tile_pool(name="vtr_ps", bufs=1, space="PSUM") as vtr_pool:
                logits_g = [
                    logits_pool.tile(
                        [P, 3, NPC, J], f32, tag=f"lg{g}", name=f"logits{g}"
                    )
                    for g in range(3)
                ]
                # s2a/s2b share one bank: only s2a's first matmul uses
                # start=True (clearing the whole bank); s2b accumulates onto
                # the cleared half without its own start.
                s2 = s2_pool.tile([P, 2, 2 * L], f32, tag="s2")

                oc = None
                for t in range(ITERS):
                    # ---------- probs folded into xs = x/sum_j exp(logits) ----------
                    if t == 0:
                        # uniform probs: s[n,j,l] = (1/J) sum_i x[n,i,l] for all j
                        for h in range(2):
                            nc.tensor.matmul(
                                s2[:, h],
                                colones[:],
                                xqs_b[:, 2 * h : 2 * h + 2].rearrange(
                                    "p n l -> p (n l)"
                                ),
                                start=(h == 0),
                                stop=(h == 1),
                                skip_group_check=True,
                            )
                    else:
                        # slab-grouped softmax: exp of slab g depends only on
                        # logits_g, i.e. on 3 of the previous iteration's 9 (b)
                        # matmuls -> overlaps with the rest of (b)
                        e_t = work.tile([P, Q, NPC, J], mmdt, tag="e")
                        d_t = work.tile([P, Q, NPC], f32, tag="d")
                        r_t = work.tile([P, Q, NPC], f32, tag="r")
                        xs = work.tile([P, Q, NPC, L], mmdt, tag="xs")
                        for g in range(3):
                            sl = slice(3 * g, 3 * g + 3)
                            nc.scalar.activation(
                                e_t[:, sl].rearrange("p q n j -> p (q n j)"),
                                logits_g[g][:].rearrange("p q n j -> p (q n j)"),
                                AF.Exp,
                            )
                            nc.vector.tensor_reduce(
                                d_t[:, sl],
                                e_t[:, sl],
                                axis=AX.X,
                                op=ALU.add,
                            )
                            nc.vector.reciprocal_approx_fast(r_t[:, sl], d_t[:, sl])
                            # last slab's xs is on the critical path into (a):
                            # vector fills its own idle gap there; earlier
                            # slabs go to gpsimd and hide behind later reduces
                            xs_eng = nc.vector if g == 2 else nc.gpsimd
                            xs_eng.tensor_mul(
                                xs[:, sl],
                                xpad[:, sl, :, 0:L],
                                r_t[:, sl].unsqueeze(3).broadcast_to((P, 3, NPC, L)),
                            )
                            # ------- (a): s2[h][(nh,j),(nh',l)] = sum_i e*xs -------
                            for qq in range(3 * g, 3 * g + 3):
                                for h in range(2):
                                    nc.tensor.matmul(
                                        s2[:, h],
                                        e_t[:, qq, 2 * h : 2 * h + 2, :].rearrange(
                                            "p n j -> p (n j)"
                                        ),
                                        xs[:, qq, 2 * h : 2 * h + 2, :].rearrange(
                                            "p n l -> p (n l)"
                                        ),
                                        start=(qq == 0 and h == 0),
                                        stop=(qq == Q - 1 and h == 1),
                                        skip_group_check=True,
                                    )

                    # extract diag blocks -> s_sb[(m j), h, l]: sample 2m+h
                    # (two same-partition copies; bf16 for DVE 2x)
                    s_sb = work.tile([P, 2, L], mmdt, tag="s_sb")
                    nc.scalar.copy(s_sb[0:J], s2[0:J, :, 0:L])
                    nc.vector.tensor_copy(s_sb[J:P], s2[J:P, :, L : 2 * L])

                    # ---------- u[(m j), h, k] = sum_l w[j,k,l] * s[(m j),h,l] ----------
                    pt = work.tile([P, 2, K, L], mmdt, tag="pt")
                    nc.vector.tensor_mul(
                        pt,
                        w_b[:].unsqueeze(1).broadcast_to((P, 2, K, L)),
                        s_sb[:].unsqueeze(2).broadcast_to((P, 2, K, L)),
                    )
                    u_t = work.tile([P, 2, K], f32, tag="u")
                    nc.vector.reduce_sum(u_t, pt, axis=AX.X)

                    # ---------- squash scale feed first: sq = sum_k u^2 ----------
                    # (Ln/Exp on ACT then overlap the v-path muls on vector)
                    u2 = work.tile([P, 2, K], f32, tag="u2")
                    nc.vector.tensor_mul(u2, u_t, u_t)
                    sq = work.tile([P, 2], f32, tag="sq")
                    nc.vector.reduce_sum(sq, u2, axis=AX.X)
                    rt = work.tile([P, 2], f32, tag="rt")  # sqrt(sq+eps)
                    nc.scalar.activation(rt, sq, AF.Ln, bias=eps_t[:])
                    nc.scalar.activation(rt, rt, AF.Exp, scale=0.5)

                    # ---------- v path from u (runs while squash scale computes):
                    # vT_u[(m j), h, l] = sum_k wT[j,l,k] * u[(m j),h,k]
                    if t != ITERS - 1:
                        qt = work.tile([P, 2, L, K], mmdt, tag="qt")
                        nc.vector.tensor_mul(
                            qt,
                            wT_b[:].unsqueeze(1).broadcast_to((P, 2, L, K)),
                            u_t[:].unsqueeze(2).broadcast_to((P, 2, L, K)),
                        )
                        vT_u = work.tile([P, 2, L], f32, tag="vT_u")
                        nc.vector.reduce_sum(vT_u, qt, axis=AX.X)

                    # scl = sq/((1+sq)*sqrt(sq+eps))
                    sp1 = work.tile([P, 2], f32, tag="sp1")  # (1+sq)*sqrt
                    nc.vector.scalar_tensor_tensor(
                        sp1, sq, 1.0, rt, op0=ALU.add, op1=ALU.mult
                    )
                    rp = work.tile([P, 2], f32, tag="rp")
                    nc.vector.reciprocal_approx_fast(rp, sp1)
                    scl = work.tile([P, 2], f32, tag="scl")
                    nc.vector.tensor_mul(scl, sq, rp)

                    if t == ITERS - 1:
                        oc = work.tile([P, 2, K], f32, tag="oc")
                        nc.vector.tensor_mul(
                            oc, u_t, scl[:].unsqueeze(2).broadcast_to((P, 2, K))
                        )
                        break

                    # ---------- v = scl * vT_u into the padded layout ----------
                    # row half m holds samples n' = {m, m+2} at col slots
                    # 32n'..32n'+8 -> strided slot AP, one write per m
                    vpv = vT_pad[:].rearrange("p (h m) x -> p m h x", h=2)
                    for m in range(2):
                        nc.vector.tensor_mul(
                            vpv[J * m : J * (m + 1), m, :, 0:L],
                            vT_u[J * m : J * (m + 1)],
                            scl[J * m : J * (m + 1)]
                            .unsqueeze(2)
                            .broadcast_to((J, 2, L)),
                        )

                    # one PE transpose: sample n'=2h+m lands at
                    # vtr[32n'+l, 64m+j] -- all 4 vblk copies same-partition
                    vtr = vtr_pool.tile([P, P], mmdt, tag="vtr")
                    nc.tensor.transpose(
                        vtr, vT_pad[:].rearrange("p n x -> p (n x)"), id_t
                    )
                    veng = [nc.scalar.copy, nc.vector.tensor_copy,
                            nc.scalar.copy, nc.vector.tensor_copy]
                    for np_ in range(NPC):
                        m = np_ % 2
                        src = vtr[32 * np_ : 32 * np_ + 8, J * m : J * (m + 1)]
                        dst = vblk[32 * np_ : 32 * np_ + 8, J * np_ : J * (np_ + 1)]
                        veng[np_](dst, src)

                    # ---------- (b): logits_g[p, s, n', j] += sum_l x * v ----------
                    for g in range(3):
                        for s in range(3):
                            qq = 3 * g + s
                            nc.tensor.matmul(
                                logits_g[g][:, s].rearrange("p n j -> p (n j)"),
                                xT_sb[:, qq, :],
                                vblk[:],
                                start=(t == 0 and s % 2 == 0),
                                stop=(t == ITERS - 2 and s >= 1),
                                skip_group_check=True,
                            )

                # ---------- output: o_d[n=2m+h, j, k] = oc[(m j), h, k] ----------
                # sample n = 2m + h lives at oc[64m + j, h, k]
                nc.sync.dma_start(
                    out=o_d[0:2].rearrange("h j k -> j h k"), in_=oc[0:J]
                )
                nc.scalar.dma_start(
                    out=o_d[2:4].rearrange("h j k -> j h k"), in_=oc[J:P]
                )

    nc.finalize()
    return nc


def kernel(x, weight):
    global LAST_RESULT
    from concourse.bass_utils import run_bass_kernel_spmd

    if "nc" not in _cache:
        _cache["nc"] = _build()
    nc = _cache["nc"]

    x = np.ascontiguousarray(np.asarray(x, dtype=np.float32))
    weight = np.ascontiguousarray(np.asarray(weight, dtype=np.float32))

    in_maps = [
        {"x": x[c * NPC : (c + 1) * NPC], "weight": weight} for c in range(NCORES)
    ]
    last_exc = None
    for attempt in range(3):
        try:
            res = run_bass_kernel_spmd(nc, in_maps, core_ids=list(range(NCORES)))
            break
        except Exception as e:
            last_exc = e
            import time

            time.sleep(5 * (attempt + 1))
    else:
        raise last_exc
    LAST_RESULT = res
    return np.concatenate([r["out"] for r in res.results], axis=0)
